# revision 20
# baseline (speedup 1.0000x reference)
"""ConvLSTM cell kernel for Trainium2 (8 NeuronCores), v3.

Sharding: data-parallel over batch B=4 x spatial split of H=64 into 2 halves
(8 shards). The recurrence prevents sharding T. Each core computes its half
with a shrinking row margin (47-t rows at step t) so no cross-core
communication is needed. Bottom halves are row-flipped on the host so a
single SPMD program serves all 8 cores.

v3 design:
- conv h2h runs in fp8e4 with MatmulPerfMode.DoubleRow: the h state lives in
  SBUF as [128, 2, 56, 66] fp8 planes: j0 = [hpad; hpad+1row] across
  partition halves, j1 = [hpad+2rows; zeros]. One DR matmul per dx column
  covers taps (dy0,dx),(dy1,dx),(dy2,dx) at K-effective 256 -> the 3x3 conv
  is 3 DR matmuls per 128-wide oc tile, plus 1 bf16 identity matmul that
  adds x into PSUM. Weights are scaled x8 and x x16 (h planes carry 2h) to
  keep fp8 operands normalized; the ACT scale divides by 16.
- gates: tile0 = [f; i] via ACT Sigmoid (scale 1/16) from PSUM;
  tile1 = [s_o; g] via ACT Tanh with per-partition scale [0.5/16 | 1/16].
- state chain on DVE: a=f*c, b=i*g, c=a+b (2x tensor_tensor), tc=tanh(c)
  (ACT), Hs=(s_o+1)*tc = 2h (scalar_tensor_tensor).
- h fp8 planes: gpsimd cast-DMA (bf16->fp8) into hpad, then two fp8 DMA
  row-shift copies for the +1row/+2row planes (all off the compute engines).
- Everything is processed in 16-row pieces so the recurrence tail pipelines
  under the PE period; PSUM per (tau,piece) = 2 banks.
- Host multiplies the output by 0.5 (kernel returns Hs = 2h).
"""

import sys

sys.path.insert(0, "/opt/trn_rl_repo")

import numpy as np
from ml_dtypes import bfloat16, float8_e4m3fn

HIDDEN = 64
T_STEPS = 16
B = 4
H = 64
W = 64
ROWS = 48        # per-core x rows (32 owned + 16 margin)
OWN = 32
WP = 80          # padded row width (j-plane stride 80 B stays 16-aligned)
PR = 50          # fp8 plane rows
XCOLS = ROWS * W
ZBYTES = PR * 2 * WP  # 8000

_CACHE = {}


def _build_nc():
    from concourse import bacc, mybir
    from concourse.tile import TileContext

    dt = mybir.dt
    Alu = mybir.AluOpType
    Act = mybir.ActivationFunctionType
    DR = mybir.MatmulPerfMode.DoubleRow

    nc = bacc.Bacc(None, target_bir_lowering=False)

    x_in = nc.dram_tensor("x", [T_STEPS, 2, 128, XCOLS], dt.bfloat16,
                          kind="ExternalInput")
    w_in = nc.dram_tensor("wdr", [128, 6, 2, 128], dt.float8e4,
                          kind="ExternalInput")
    id_in = nc.dram_tensor("ident", [128, 128], dt.bfloat16,
                           kind="ExternalInput")
    sc_in = nc.dram_tensor("scog", [128, 1], dt.float32,
                           kind="ExternalInput")
    z_in = nc.dram_tensor("zeros", [128, ZBYTES], dt.uint8,
                          kind="ExternalInput")
    hout = nc.dram_tensor("hout", [T_STEPS, 64, OWN * W], dt.bfloat16,
                          kind="ExternalOutput")

    with TileContext(nc) as tc:
        with (
            tc.tile_pool(name="const", bufs=1) as cpool,
            tc.tile_pool(name="state", bufs=1) as spool,
            tc.tile_pool(name="xload", bufs=3) as xpool,
            tc.tile_pool(name="gate", bufs=2) as gpool,
            tc.tile_pool(name="work", bufs=3) as wpool,
            tc.tile_pool(name="ps", bufs=1, space="PSUM") as psp,
        ):
            w_sb = cpool.tile([128, 6, 2, 128], dt.float8e4, tag="w")
            id_sb = cpool.tile([128, 128], dt.bfloat16, tag="id")
            sc_sb = cpool.tile([128, 1], dt.float32, tag="sc")
            nc.sync.dma_start(out=w_sb[:], in_=w_in[:])
            nc.sync.dma_start(out=id_sb[:], in_=id_in[:])
            nc.sync.dma_start(out=sc_sb[:], in_=sc_in[:])

            # h fp8 planes, row-major with the 2 j-planes interleaved per row
            # so a DR matmul's byte interval only spans the rows it reads:
            # hb[p<64, k, 0] = hpad[k]; hb[p>=64, k, 0] = hpad[k+1];
            # hb[p<64, k, 1] = hpad[k+2]; hb[p>=64, k, 1] = zeros.
            hb = [
                spool.tile([128, PR, 2, WP], dt.float8e4, tag="hb0", name="hb0"),
                spool.tile([128, PR, 2, WP], dt.float8e4, tag="hb1", name="hb1"),
            ]
            cst = spool.tile([64, XCOLS], dt.bfloat16, tag="cst")
            nc.sync.dma_start(
                out=hb[0][:].rearrange("p a b c -> p (a b c)").bitcast(dt.uint8),
                in_=z_in[:])
            nc.sync.dma_start(
                out=hb[1][:].rearrange("p a b c -> p (a b c)").bitcast(dt.uint8),
                in_=z_in[:])
            nc.scalar.dma_start(out=cst[:].bitcast(dt.uint8),
                                in_=z_in[0:64, : XCOLS * 2])

            ps = [psp.tile([128, 2048], dt.float32, tag=f"ps{i}", name=f"ps{i}")
                  for i in range(2)]

            # Hoist all x loads: the sync queue carries only x DMAs, issued
            # ahead; the xpool bufs=3 rotation throttles to a 3-step rolling
            # prefetch so x(t+1) never waits on step t's tail.
            xtiles = []
            for t in range(T_STEPS):
                R = 47 - t
                pair = []
                for tau in range(2):
                    xti = xpool.tile([128, XCOLS], dt.bfloat16,
                                     tag=f"x{tau}", name=f"x{tau}")
                    nc.sync.dma_start(out=xti[:, : R * W],
                                      in_=x_in[t, tau][:, : R * W])
                    pair.append(xti)
                xtiles.append(pair)

            for t in range(T_STEPS):
                R = 47 - t
                hbr = hb[t % 2]
                hbw = hb[(t + 1) % 2]
                last = t == T_STEPS - 1
                xt = xtiles[t]

                fi = gpool.tile([128, XCOLS], dt.bfloat16, tag="fi", name="fi")
                og = gpool.tile([128, XCOLS], dt.bfloat16, tag="og", name="og")
                hcomp = gpool.tile([64, XCOLS], dt.bfloat16, tag="hc",
                                   name="hc")

                nrows = OWN if last else R
                bounds = [0, 16, 32, nrows]
                pieces = [(a, min(b, nrows)) for a, b in
                          zip(bounds[:-1], bounds[1:]) if a < nrows]
                pbases = [0, 1024, 0]

                def mm_subs(rows):
                    subs = []
                    q = 0
                    while q < rows:
                        sr = min(8, rows - q)
                        subs.append((q, sr))
                        q += sr
                    return subs

                def id_mms(pi, r0, r1):
                    pbase = pbases[pi]
                    for tau in range(2):
                        for (q, sr) in mm_subs(r1 - r0):
                            nc.tensor.matmul(
                                ps[tau][:, pbase + q * W:
                                        pbase + (q + sr) * W],
                                lhsT=id_sb[:],
                                rhs=xt[tau][:, (r0 + q) * W:
                                            (r0 + q + sr) * W],
                                start=True, stop=(t == 0))

                def dr_mms(pi, r0, r1):
                    pbase = pbases[pi]
                    for tau in range(2):
                        for d in range(3):
                            wap = w_sb[:, tau * 3 + d, :, :]
                            for (q, sr) in mm_subs(r1 - r0):
                                rhs = hbr[:, r0 + q: r0 + q + sr, :,
                                          d: d + W].transpose([0, 2, 1, 3])
                                nc.tensor.matmul(
                                    ps[tau][:, pbase + q * W:
                                            pbase + (q + sr) * W],
                                    lhsT=wap,
                                    rhs=rhs,
                                    start=False, stop=(d == 2),
                                    perf_mode=DR)

                def gates_act(pi, r0, r1):
                    pbase = pbases[pi]
                    ncols = (r1 - r0) * W
                    seg = slice(r0 * W, r1 * W)
                    pseg = slice(pbase, pbase + ncols)
                    nc.scalar.activation(og[:, seg], ps[1][:, pseg],
                                         Act.Tanh, scale=sc_sb[:])
                    nc.scalar.activation(fi[:, seg], ps[0][:, pseg],
                                         Act.Sigmoid, scale=1.0 / 16)

                def chain(pi, r0, r1):
                    rows = r1 - r0
                    ncols = rows * W
                    at = wpool.tile([64, 1024], dt.bfloat16, tag="at",
                                    name="at")
                    bt = wpool.tile([64, 1024], dt.bfloat16, tag="bt",
                                    name="bt")
                    ot = wpool.tile([64, 1024], dt.bfloat16, tag="ot",
                                    name="ot")
                    tct = wpool.tile([64, 1024], dt.bfloat16, tag="tct",
                                     name="tct")
                    seg = slice(r0 * W, r1 * W)
                    # o = 0.5*s_o + 0.5 (4x tensor_scalar; only needs og)
                    nc.vector.tensor_scalar(ot[:, :ncols], og[0:64, seg],
                                            0.5, 0.5, Alu.mult, Alu.add)
                    nc.vector.tensor_tensor(at[:, :ncols], fi[0:64, seg],
                                            cst[:, seg], Alu.mult)
                    nc.vector.tensor_tensor(bt[:, :ncols], fi[64:128, seg],
                                            og[64:128, seg], Alu.mult)
                    nc.vector.tensor_tensor(cst[:, seg], at[:, :ncols],
                                            bt[:, :ncols], Alu.add)
                    nc.scalar.activation(tct[:, :ncols], cst[:, seg], Act.Tanh)
                    nc.vector.tensor_tensor(hcomp[:, seg], ot[:, :ncols],
                                            tct[:, :ncols], Alu.mult)

                    if not last:
                        # fp8 shadow planes for the next step's conv
                        src = hcomp[:, seg].rearrange(
                            "p (r c) -> p r c", r=rows)
                        nc.vector.tensor_copy(
                            hbw[0:64, 1 + r0: 1 + r1, 0, 1: 1 + W], src)
                        # +1row half: dest row k = hpad[k+1]
                        nc.vector.tensor_copy(
                            hbw[64:128, r0: r1, 0, :],
                            hbw[0:64, 1 + r0: 1 + r1, 0, :])
                        # +2row plane (j=1, lower half): row k = hpad[k+2]
                        k0 = max(r0 - 1, 0)
                        k1 = r1 - 1
                        nc.vector.tensor_copy(
                            hbw[0:64, k0: k1, 1, :],
                            hbw[0:64, k0 + 2: k1 + 2, 0, :])

                # PE stream: id MMs for pieces 0-2 first (disjoint PSUM
                # regions, only need x — they fill the pipe while the previous
                # step's tail finishes), then DR by piece; piece 3 reuses
                # piece 0/1's PSUM banks so its id MMs come after piece 1's DR
                # (avoids an in-order-queue cycle with ACT of pieces 0/1).
                for pi, (r0, r1) in enumerate(pieces[:2]):
                    id_mms(pi, r0, r1)
                for pi, (r0, r1) in enumerate(pieces):
                    if pi >= 2:
                        id_mms(pi, r0, r1)
                    if t > 0:
                        dr_mms(pi, r0, r1)
                    gates_act(pi, r0, r1)
                    chain(pi, r0, r1)
                    if r1 == OWN:
                        nc.scalar.dma_start(out=hout[t],
                                            in_=hcomp[:, : OWN * W])

    nc.finalize()
    return nc


def _prep_inputs(x, w_h2h):
    """Per-core input maps. Cores: core = b*2 + half."""
    # tau0 tile = [f; i], tau1 tile = [o; g]
    perm = np.concatenate([np.arange(64, 128), np.arange(0, 64),
                           np.arange(128, 192), np.arange(192, 256)])
    # kernel computes 16*z in PSUM: w*16 against h-planes, identity*16 for x
    w_perm = w_h2h.astype(np.float32)[perm] * 16.0  # [256, 64, 3, 3]

    def pack_w(wp):
        out = np.zeros((128, 6, 2, 128), np.float32)
        for tau in range(2):
            blk = wp[tau * 128: (tau + 1) * 128]  # [128 oc, 64 ic, 3, 3]
            for d in range(3):
                s = tau * 3 + d
                out[0:64, s, 0, :] = blk[:, :, 0, d].T
                out[64:128, s, 0, :] = blk[:, :, 1, d].T
                out[0:64, s, 1, :] = blk[:, :, 2, d].T
        return np.clip(out, -240, 240).astype(float8_e4m3fn)

    w_top = pack_w(w_perm)
    w_bot = pack_w(w_perm[:, :, ::-1, :])
    ident = (np.eye(128, dtype=np.float32) * 16.0).astype(bfloat16)
    scog = np.concatenate([np.full((64, 1), 0.5 / 16, np.float32),
                           np.full((64, 1), 1.0 / 16, np.float32)])
    zeros = np.zeros((128, ZBYTES), np.uint8)

    xp = x[:, :, perm]  # [T, B, 256, H, W]
    in_maps = []
    for b in range(B):
        for half in range(2):
            if half == 0:
                xs = xp[:, b, :, 0:ROWS, :]
            else:
                xs = xp[:, b, :, H - ROWS:, :][:, :, ::-1, :]
            xs = np.ascontiguousarray(xs).astype(bfloat16)
            xs = xs.reshape(T_STEPS, 2, 128, XCOLS)
            in_maps.append({
                "x": xs,
                "wdr": w_top if half == 0 else w_bot,
                "ident": ident,
                "scog": scog,
                "zeros": zeros,
            })
    return in_maps


def kernel(x, w_h2h):
    from concourse import bass_utils

    if "nc" not in _CACHE:
        _CACHE["nc"] = _build_nc()
    nc = _CACHE["nc"]

    in_maps = _prep_inputs(np.asarray(x), np.asarray(w_h2h))
    res = bass_utils.run_bass_kernel_spmd(nc, in_maps,
                                          core_ids=list(range(8)),
                                          **_CACHE.get("run_kwargs", {}))
    _CACHE["last_results"] = res

    out = np.zeros((T_STEPS, B, HIDDEN, H, W), np.float32)
    for b in range(B):
        for half in range(2):
            core = b * 2 + half
            hs = res.results[core]["hout"].astype(np.float32)
            hs = hs.reshape(T_STEPS, HIDDEN, OWN, W)
            if half == 0:
                out[:, b, :, 0:OWN, :] = hs
            else:
                out[:, b, :, OWN:, :] = hs[:, :, ::-1, :]
    return out


# revision 21
# speedup vs baseline: 1.1975x; 1.1975x over previous
"""ConvLSTM cell kernel for Trainium2 (8 NeuronCores), v3.

Sharding: data-parallel over batch B=4 x spatial split of H=64 into 2 halves
(8 shards). The recurrence prevents sharding T. Each core computes its half
with a shrinking row margin (47-t rows at step t) so no cross-core
communication is needed. Bottom halves are row-flipped on the host so a
single SPMD program serves all 8 cores.

v3 design:
- conv h2h runs in fp8e4 with MatmulPerfMode.DoubleRow: the h state lives in
  SBUF as [128, 2, 56, 66] fp8 planes: j0 = [hpad; hpad+1row] across
  partition halves, j1 = [hpad+2rows; zeros]. One DR matmul per dx column
  covers taps (dy0,dx),(dy1,dx),(dy2,dx) at K-effective 256 -> the 3x3 conv
  is 3 DR matmuls per 128-wide oc tile, plus 1 bf16 identity matmul that
  adds x into PSUM. Weights are scaled x8 and x x16 (h planes carry 2h) to
  keep fp8 operands normalized; the ACT scale divides by 16.
- gates: tile0 = [f; i] via ACT Sigmoid (scale 1/16) from PSUM;
  tile1 = [s_o; g] via ACT Tanh with per-partition scale [0.5/16 | 1/16].
- state chain on DVE: a=f*c, b=i*g, c=a+b (2x tensor_tensor), tc=tanh(c)
  (ACT), Hs=(s_o+1)*tc = 2h (scalar_tensor_tensor).
- h fp8 planes: gpsimd cast-DMA (bf16->fp8) into hpad, then two fp8 DMA
  row-shift copies for the +1row/+2row planes (all off the compute engines).
- Everything is processed in 16-row pieces so the recurrence tail pipelines
  under the PE period; PSUM per (tau,piece) = 2 banks.
- Host multiplies the output by 0.5 (kernel returns Hs = 2h).
"""

import sys

sys.path.insert(0, "/opt/trn_rl_repo")

import numpy as np
from ml_dtypes import bfloat16, float8_e4m3fn

HIDDEN = 64
T_STEPS = 16
B = 4
H = 64
W = 64
ROWS = 48        # per-core x rows (32 owned + 16 margin)
OWN = 32
WP = 80          # padded row width (j-plane stride 80 B stays 16-aligned)
PR = 50          # fp8 plane rows
XCOLS = ROWS * W
ZBYTES = PR * 2 * WP  # 8000

_CACHE = {}


def _build_nc():
    from concourse import bacc, mybir
    from concourse.tile import TileContext

    dt = mybir.dt
    Alu = mybir.AluOpType
    Act = mybir.ActivationFunctionType
    DR = mybir.MatmulPerfMode.DoubleRow

    nc = bacc.Bacc(None, target_bir_lowering=False)

    x_in = nc.dram_tensor("x", [T_STEPS, 2, 128, XCOLS], dt.bfloat16,
                          kind="ExternalInput")
    w_in = nc.dram_tensor("wdr", [128, 6, 2, 128], dt.float8e4,
                          kind="ExternalInput")
    id_in = nc.dram_tensor("ident", [128, 128], dt.bfloat16,
                           kind="ExternalInput")
    sc_in = nc.dram_tensor("scog", [128, 1], dt.float32,
                           kind="ExternalInput")
    z_in = nc.dram_tensor("zeros", [128, ZBYTES], dt.uint8,
                          kind="ExternalInput")
    hout = nc.dram_tensor("hout", [T_STEPS, 64, OWN * W], dt.bfloat16,
                          kind="ExternalOutput")

    with TileContext(nc) as tc:
        with (
            tc.tile_pool(name="const", bufs=1) as cpool,
            tc.tile_pool(name="state", bufs=1) as spool,
            tc.tile_pool(name="xload", bufs=3) as xpool,
            tc.tile_pool(name="gate", bufs=2) as gpool,
            tc.tile_pool(name="work", bufs=3) as wpool,
            tc.tile_pool(name="ps", bufs=1, space="PSUM") as psp,
        ):
            w_sb = cpool.tile([128, 6, 2, 128], dt.float8e4, tag="w")
            id_sb = cpool.tile([128, 128], dt.bfloat16, tag="id")
            sc_sb = cpool.tile([128, 1], dt.float32, tag="sc")
            nc.sync.dma_start(out=w_sb[:], in_=w_in[:])
            nc.sync.dma_start(out=id_sb[:], in_=id_in[:])
            nc.sync.dma_start(out=sc_sb[:], in_=sc_in[:])

            # h fp8 planes, row-major with the 2 j-planes interleaved per row
            # so a DR matmul's byte interval only spans the rows it reads:
            # hb[p<64, k, 0] = hpad[k]; hb[p>=64, k, 0] = hpad[k+1];
            # hb[p<64, k, 1] = hpad[k+2]; hb[p>=64, k, 1] = zeros.
            hb = [
                spool.tile([128, PR, 2, WP], dt.float8e4, tag="hb0", name="hb0"),
                spool.tile([128, PR, 2, WP], dt.float8e4, tag="hb1", name="hb1"),
            ]
            cst = spool.tile([64, XCOLS], dt.bfloat16, tag="cst")
            nc.sync.dma_start(
                out=hb[0][:].rearrange("p a b c -> p (a b c)").bitcast(dt.uint8),
                in_=z_in[:])
            nc.sync.dma_start(
                out=hb[1][:].rearrange("p a b c -> p (a b c)").bitcast(dt.uint8),
                in_=z_in[:])
            nc.scalar.dma_start(out=cst[:].bitcast(dt.uint8),
                                in_=z_in[0:64, : XCOLS * 2])

            ps = [psp.tile([128, 2048], dt.float32, tag=f"ps{i}", name=f"ps{i}")
                  for i in range(2)]

            # Hoist all x loads: the sync queue carries only x DMAs, issued
            # ahead; the xpool bufs=3 rotation throttles to a 3-step rolling
            # prefetch so x(t+1) never waits on step t's tail.
            xtiles = []
            for t in range(T_STEPS):
                R = 47 - t
                pair = []
                for tau in range(2):
                    xti = xpool.tile([128, XCOLS], dt.bfloat16,
                                     tag=f"x{tau}", name=f"x{tau}")
                    nc.sync.dma_start(out=xti[:, : R * W],
                                      in_=x_in[t, tau][:, : R * W])
                    pair.append(xti)
                xtiles.append(pair)

            for t in range(T_STEPS):
                R = 47 - t
                hbr = hb[t % 2]
                hbw = hb[(t + 1) % 2]
                last = t == T_STEPS - 1
                xt = xtiles[t]

                fi = gpool.tile([128, XCOLS], dt.bfloat16, tag="fi", name="fi")
                og = gpool.tile([128, XCOLS], dt.bfloat16, tag="og", name="og")
                hcomp = gpool.tile([64, XCOLS], dt.bfloat16, tag="hc",
                                   name="hc")

                nrows = OWN if last else R
                bounds = [0, 16, 32, nrows]
                pieces = [(a, min(b, nrows)) for a, b in
                          zip(bounds[:-1], bounds[1:]) if a < nrows]
                pbases = [0, 1024, 0]

                def mm_subs(rows):
                    subs = []
                    q = 0
                    while q < rows:
                        sr = min(8, rows - q)
                        subs.append((q, sr))
                        q += sr
                    return subs

                def id_mms(pi, r0, r1):
                    pbase = pbases[pi]
                    for tau in range(2):
                        for (q, sr) in mm_subs(r1 - r0):
                            nc.tensor.matmul(
                                ps[tau][:, pbase + q * W:
                                        pbase + (q + sr) * W],
                                lhsT=id_sb[:],
                                rhs=xt[tau][:, (r0 + q) * W:
                                            (r0 + q + sr) * W],
                                start=True, stop=(t == 0))

                def dr_mms(pi, r0, r1):
                    pbase = pbases[pi]
                    for tau in range(2):
                        for d in range(3):
                            wap = w_sb[:, tau * 3 + d, :, :]
                            for (q, sr) in mm_subs(r1 - r0):
                                rhs = hbr[:, r0 + q: r0 + q + sr, :,
                                          d: d + W].transpose([0, 2, 1, 3])
                                nc.tensor.matmul(
                                    ps[tau][:, pbase + q * W:
                                            pbase + (q + sr) * W],
                                    lhsT=wap,
                                    rhs=rhs,
                                    start=False, stop=(d == 2),
                                    perf_mode=DR)

                def gates_act(pi, r0, r1):
                    pbase = pbases[pi]
                    ncols = (r1 - r0) * W
                    seg = slice(r0 * W, r1 * W)
                    pseg = slice(pbase, pbase + ncols)
                    nc.scalar.activation(og[:, seg], ps[1][:, pseg],
                                         Act.Tanh, scale=sc_sb[:])
                    nc.scalar.activation(fi[:, seg], ps[0][:, pseg],
                                         Act.Sigmoid, scale=1.0 / 16)

                def chain(pi, r0, r1):
                    rows = r1 - r0
                    ncols = rows * W
                    at = wpool.tile([64, 1024], dt.bfloat16, tag="at",
                                    name="at")
                    bt = wpool.tile([64, 1024], dt.bfloat16, tag="bt",
                                    name="bt")
                    ot = wpool.tile([64, 1024], dt.bfloat16, tag="ot",
                                    name="ot")
                    tct = wpool.tile([64, 1024], dt.bfloat16, tag="tct",
                                     name="tct")
                    seg = slice(r0 * W, r1 * W)
                    # o = 0.5*s_o + 0.5 (4x tensor_scalar; only needs og)
                    nc.vector.tensor_scalar(ot[:, :ncols], og[0:64, seg],
                                            0.5, 0.5, Alu.mult, Alu.add)
                    nc.vector.tensor_tensor(at[:, :ncols], fi[0:64, seg],
                                            cst[:, seg], Alu.mult)
                    nc.vector.tensor_tensor(bt[:, :ncols], fi[64:128, seg],
                                            og[64:128, seg], Alu.mult)
                    nc.vector.tensor_tensor(cst[:, seg], at[:, :ncols],
                                            bt[:, :ncols], Alu.add)
                    nc.scalar.activation(tct[:, :ncols], cst[:, seg], Act.Tanh)
                    nc.vector.tensor_tensor(hcomp[:, seg], ot[:, :ncols],
                                            tct[:, :ncols], Alu.mult)

                    if not last:
                        # fp8 shadow planes for the next step's conv
                        src = hcomp[:, seg].rearrange(
                            "p (r c) -> p r c", r=rows)
                        nc.vector.tensor_copy(
                            hbw[0:64, 1 + r0: 1 + r1, 0, 1: 1 + W], src)
                        # +1row half: dest row k = hpad[k+1]
                        nc.vector.tensor_copy(
                            hbw[64:128, r0: r1, 0, :].bitcast(dt.uint16),
                            hbw[0:64, 1 + r0: 1 + r1, 0, :].bitcast(dt.uint16))
                        # +2row plane (j=1, lower half): row k = hpad[k+2]
                        k0 = max(r0 - 1, 0)
                        k1 = r1 - 1
                        nc.vector.tensor_copy(
                            hbw[0:64, k0: k1, 1, :].bitcast(dt.uint16),
                            hbw[0:64, k0 + 2: k1 + 2, 0, :].bitcast(dt.uint16))

                # PE stream: id MMs for pieces 0-2 first (disjoint PSUM
                # regions, only need x — they fill the pipe while the previous
                # step's tail finishes), then DR by piece; piece 3 reuses
                # piece 0/1's PSUM banks so its id MMs come after piece 1's DR
                # (avoids an in-order-queue cycle with ACT of pieces 0/1).
                for pi, (r0, r1) in enumerate(pieces[:2]):
                    id_mms(pi, r0, r1)
                for pi, (r0, r1) in enumerate(pieces):
                    if pi >= 2:
                        id_mms(pi, r0, r1)
                    if t > 0:
                        dr_mms(pi, r0, r1)
                    gates_act(pi, r0, r1)
                    chain(pi, r0, r1)
                    if r1 == OWN:
                        nc.scalar.dma_start(out=hout[t],
                                            in_=hcomp[:, : OWN * W])

    nc.finalize()
    return nc


def _prep_inputs(x, w_h2h):
    """Per-core input maps. Cores: core = b*2 + half."""
    # tau0 tile = [f; i], tau1 tile = [o; g]
    perm = np.concatenate([np.arange(64, 128), np.arange(0, 64),
                           np.arange(128, 192), np.arange(192, 256)])
    # kernel computes 16*z in PSUM: w*16 against h-planes, identity*16 for x
    w_perm = w_h2h.astype(np.float32)[perm] * 16.0  # [256, 64, 3, 3]

    def pack_w(wp):
        out = np.zeros((128, 6, 2, 128), np.float32)
        for tau in range(2):
            blk = wp[tau * 128: (tau + 1) * 128]  # [128 oc, 64 ic, 3, 3]
            for d in range(3):
                s = tau * 3 + d
                out[0:64, s, 0, :] = blk[:, :, 0, d].T
                out[64:128, s, 0, :] = blk[:, :, 1, d].T
                out[0:64, s, 1, :] = blk[:, :, 2, d].T
        return np.clip(out, -240, 240).astype(float8_e4m3fn)

    w_top = pack_w(w_perm)
    w_bot = pack_w(w_perm[:, :, ::-1, :])
    ident = (np.eye(128, dtype=np.float32) * 16.0).astype(bfloat16)
    scog = np.concatenate([np.full((64, 1), 0.5 / 16, np.float32),
                           np.full((64, 1), 1.0 / 16, np.float32)])
    zeros = np.zeros((128, ZBYTES), np.uint8)

    xp = x[:, :, perm]  # [T, B, 256, H, W]
    in_maps = []
    for b in range(B):
        for half in range(2):
            if half == 0:
                xs = xp[:, b, :, 0:ROWS, :]
            else:
                xs = xp[:, b, :, H - ROWS:, :][:, :, ::-1, :]
            xs = np.ascontiguousarray(xs).astype(bfloat16)
            xs = xs.reshape(T_STEPS, 2, 128, XCOLS)
            in_maps.append({
                "x": xs,
                "wdr": w_top if half == 0 else w_bot,
                "ident": ident,
                "scog": scog,
                "zeros": zeros,
            })
    return in_maps


def kernel(x, w_h2h):
    from concourse import bass_utils

    if "nc" not in _CACHE:
        _CACHE["nc"] = _build_nc()
    nc = _CACHE["nc"]

    in_maps = _prep_inputs(np.asarray(x), np.asarray(w_h2h))
    res = bass_utils.run_bass_kernel_spmd(nc, in_maps,
                                          core_ids=list(range(8)),
                                          **_CACHE.get("run_kwargs", {}))
    _CACHE["last_results"] = res

    out = np.zeros((T_STEPS, B, HIDDEN, H, W), np.float32)
    for b in range(B):
        for half in range(2):
            core = b * 2 + half
            hs = res.results[core]["hout"].astype(np.float32)
            hs = hs.reshape(T_STEPS, HIDDEN, OWN, W)
            if half == 0:
                out[:, b, :, 0:OWN, :] = hs
            else:
                out[:, b, :, OWN:, :] = hs[:, :, ::-1, :]
    return out
